# revision 1
# baseline (speedup 1.0000x reference)
"""Trainium2 Bass kernel: causal depthwise Conv1d (K=4) + SiLU.

Reference computation (B=4, S=4096, D=2048):
    y[b, s, d] = silu( sum_k w[d, 0, k] * x[b, s-3+k, d] )   (zero-padded left)

Strategy:
  * Host: transpose x to channel-major (D, B, S), left-pad each row with
    4 zeros (row length 4100), cast to bf16, shard D across the 8
    NeuronCores (256 channels each).  Depthwise conv is channel-independent
    -> no inter-core communication.
  * Core: 8 tiles of [128, 4100].  Tiles are computed on the TensorEngine
    (diag-stationary matmuls accumulate the 4 taps in PSUM; ACT silu drains
    PSUM -> bf16) or the VectorEngine (4 tensor_scalar muls @4x + 3 adds
    @2x; ACT silu).  One tile is split between the two for balance.
    Compute/drain units are emitted in modeled completion order so the
    strict-FIFO ACT queue never head-of-line blocks (that stalls PSUM
    drain -> stalls PE -> HAM cold).
  * Host: gather, transpose back, cast to f32.
"""

import os
import sys

sys.path.insert(0, "/opt/trn_rl_repo")

import numpy as np
import ml_dtypes

N_CORES = 8
B, S, D = 4, 4096, 2048
K = 4
PAD = 4
ROW = S + PAD  # 4100
D_LOCAL = D // N_CORES  # 256
G = D_LOCAL // 128  # 2 partition groups per core

MM_N = int(os.environ.get("KERNEL_MM_N", "512"))
IN_CHUNKS = int(os.environ.get("KERNEL_IN_CHUNKS", "2"))
# columns of the split tile computed on DVE (rest go to PE)
SPLIT_DVE_COLS = int(os.environ.get("KERNEL_SPLIT_DVE", "2560"))
WARMUP_MMS = int(os.environ.get("KERNEL_WARMUP", "7"))

_CACHE = {}

# ---- cost model (ns) for emission ordering -------------------------------
PE_START = 12000.0
DVE_START = 13500.0
PE_NS_PER_COL = 4.0 / 2.4  # 4 taps x 1 col / 2.4GHz


def _dve_chain_ns(w):
    ts = 4 * (w * 0.2604 + 210.0)
    tt = 3 * (w * 0.5208 + 150.0)
    return ts + tt


def _build():
    import concourse.tile as tile
    from concourse import bacc, mybir

    nc = bacc.Bacc("TRN2", debug=False, enable_asserts=False, num_devices=N_CORES)
    bf16 = mybir.dt.bfloat16
    f32 = mybir.dt.float32

    x_ap = nc.dram_tensor("x", [G, 128, B, ROW], bf16, kind="ExternalInput").ap()
    wd_ap = nc.dram_tensor("wd", [128, G * K * 128], bf16, kind="ExternalInput").ap()
    w_ap = nc.dram_tensor("w", [128, G * K], f32, kind="ExternalInput").ap()
    out_ap = nc.dram_tensor("out", [G, 128, B, S], bf16, kind="ExternalOutput").ap()

    DVE_TILES = (1, 3)
    SPLIT_TILE = 5

    with tile.TileContext(nc) as tc:
        with (
            tc.tile_pool(name="wp", bufs=1) as wp,
            tc.tile_pool(name="xp", bufs=8) as xp,
            tc.tile_pool(name="tp", bufs=2) as tp,
            tc.tile_pool(name="cp", bufs=2) as cp,
            tc.tile_pool(name="ps", bufs=int(os.environ.get("KERNEL_PS_BUFS", "2")), space="PSUM") as ps,
            tc.tile_pool(name="yp", bufs=4) as yp,
        ):
            # weights go on the sync queue (HWDGE, fast; issuing them on
            # scalar causes a second ACT_TABLE_LOAD), interleaved into the
            # input stream below so tile0's ramp chunks land first
            wd = wp.tile([128, G * K * 128], bf16, tag="wd")
            wt = wp.tile([128, G * K], f32, tag="wt")

            def wdiag(g, k):
                c0 = (g * K + k) * 128
                return wd[:, c0 : c0 + 128]

            # HAM warmup: dummy matmuls on a zeroed stationary, gated only on
            # a gpsimd memset, keep the PE busy through the ~3.4us activity
            # window so real chunks run at 2.4GHz. Result is never read.
            if WARMUP_MMS:
                zt = wp.tile([128, MM_N], bf16, tag="zt")
                nc.gpsimd.memset(zt[:], 0)
                # same tag as real chunks: rotates through the acc buffers,
                # no extra PSUM footprint (it has no readers)
                warm = ps.tile([128, 1024], f32, tag="acc")
                for _ in range(WARMUP_MMS):
                    nc.tensor.matmul(
                        warm[:, 0:MM_N], zt[:, 0:128], zt[:],
                        start=True, stop=True,
                    )

            # input chunk DMAs on the sync queue; chunk boundaries sit 4 cols
            # past each compute-chunk boundary (a chunk reads up to lo+W+3).
            # Tile 0 streams in 4 small chunks for a fast PE ramp; tiles 0/1
            # interleave at chunk level so PE and DVE both start early.
            xts = [None] * (G * B)
            tile_bounds = {}
            for ti in range(G * B):
                if IN_CHUNKS < 2:
                    tile_bounds[ti] = [0, ROW]
                elif ti == 0:
                    tile_bounds[ti] = [0, 1028, 2052, 3076, ROW]
                else:
                    tile_bounds[ti] = [0, 2052, ROW]
            chunk_order = []
            if IN_CHUNKS == 2:
                chunk_order += [
                    "wd", "wt", (0, 0), (1, 0), (0, 1), (1, 1), (0, 2), (0, 3),
                ]
                rest = range(2, G * B)
            else:
                chunk_order += ["wd", "wt"]
                rest = range(G * B)
            for ti in rest:
                for ci in range(len(tile_bounds[ti]) - 1):
                    chunk_order.append((ti, ci))
            for item in chunk_order:
                if item == "wd":
                    nc.sync.dma_start(out=wd[:], in_=wd_ap[:])
                    continue
                if item == "wt":
                    nc.sync.dma_start(out=wt[:], in_=w_ap[:])
                    continue
                ti, ci = item
                g, b = divmod(ti, B)
                if xts[ti] is None:
                    xt = xp.tile([128, ROW], bf16, tag="xt")
                    xts[ti] = xt
                c0, c1 = tile_bounds[ti][ci], tile_bounds[ti][ci + 1]
                nc.sync.dma_start(
                    out=xts[ti][:, c0:c1], in_=x_ap[g, :, b, c0:c1]
                )

            # ---- build unit worklist with modeled completion times -------
            # kinds: "pe" (chunk: matmuls+silu+dma), "dvec" (vector chain),
            # "dves" (silu+dma for a sub-range of a finished chain)
            units = []  # (ready_ns, kind, tile_idx, lo, hi)
            t_pe = PE_START
            t_dve = DVE_START
            PE_CHUNK = int(os.environ.get("KERNEL_PE_CHUNK", "2048"))
            pe_units = []
            for ti in range(G * B):
                if ti in DVE_TILES:
                    pe_units.append(None)
                elif ti == 0 and IN_CHUNKS == 2:
                    # small first units: fast ramp to the first PSUM drain
                    pe_units.append(
                        [(0, 1024), (1024, 2048), (2048, 3072), (3072, S)]
                    )
                else:
                    lo = SPLIT_DVE_COLS if ti == SPLIT_TILE else 0
                    us = []
                    c0 = lo
                    while c0 < S:
                        c1 = min(c0 + PE_CHUNK, S)
                        us.append((c0, c1))
                        c0 = c1
                    pe_units.append(us)
            for ti in range(G * B):
                if pe_units[ti] is None:
                    continue
                for lo, hi in pe_units[ti]:
                    t_pe += (hi - lo) * PE_NS_PER_COL
                    units.append((t_pe, "pe", ti, lo, hi))

            def add_dve_chain(ti, lo, hi):
                nonlocal t_dve
                t_dve += _dve_chain_ns(hi - lo)
                units.append((t_dve, "dvec", ti, lo, hi))
                # silu/dma drains in 2048 sub-chunks, interleaved with PE
                # units by the sort so ACT never head-of-line blocks PSUM
                for i, c0 in enumerate(range(lo, hi, 2048)):
                    cw = min(2048, hi - c0)
                    units.append((t_dve + 1 + 2500 * i, "dves", ti, c0, c0 + cw))

            d0 = DVE_TILES[0]
            add_dve_chain(d0, 0, 2048)
            add_dve_chain(d0, 2048, S)
            for ti in DVE_TILES[1:]:
                add_dve_chain(ti, 0, S)
            add_dve_chain(SPLIT_TILE, 0, SPLIT_DVE_COLS)

            units.sort(key=lambda u: u[0])

            def wcol(g, k):
                return wt[:, g * K + k : g * K + k + 1]

            def emit_pe(g, b, xt, lo, hi, last):
                cw = hi - lo
                y = yp.tile([128, cw], bf16, tag="y")
                acc = ps.tile([128, cw], f32, tag="acc")
                for k in range(K):
                    for n0 in range(0, cw, MM_N):
                        xlo = lo + n0 + 1 + k
                        nw = min(MM_N, cw - n0)
                        nc.tensor.matmul(
                            acc[:, n0 : n0 + nw],
                            wdiag(g, k),
                            xt[:, xlo : xlo + nw],
                            start=(k == 0),
                            stop=(k == K - 1),
                        )
                if last:
                    # fine-grained drain: silu+dma interleaved on the ACT
                    # FIFO so the first sub-chunk streams out while the
                    # second is still activating
                    for s0 in range(0, cw, 1024):
                        sw = min(1024, cw - s0)
                        nc.scalar.activation(
                            out=y[:, s0 : s0 + sw],
                            in_=acc[:, s0 : s0 + sw],
                            func=mybir.ActivationFunctionType.Silu,
                        )
                        nc.scalar.dma_start(
                            out=out_ap[g, :, b, lo + s0 : lo + s0 + sw],
                            in_=y[:, s0 : s0 + sw],
                        )
                else:
                    nc.scalar.activation(
                        out=y[:], in_=acc[:], func=mybir.ActivationFunctionType.Silu
                    )
                    nc.gpsimd.dma_start(out=out_ap[g, :, b, lo:hi], in_=y[:])

            cbufs = {}  # (ti, 2048-chunk lo) -> (c tile, chain lo)

            def emit_dve_chain(g, b, ti, xt, lo, hi):
                W = hi - lo
                ts = []
                for k in range(K):
                    t = tp.tile([128, W], bf16, tag=f"t{k % 2}")
                    nc.vector.tensor_scalar_mul(
                        t[:], xt[:, lo + 1 + k : lo + 1 + k + W], wcol(g, k)
                    )
                    ts.append(t)
                p0 = cp.tile([128, W], bf16, tag="p0")
                nc.vector.tensor_add(p0[:], ts[0][:], ts[1][:])
                p1 = cp.tile([128, W], bf16, tag="p1")
                nc.vector.tensor_add(p1[:], ts[2][:], ts[3][:])
                c = cp.tile([128, W], bf16, tag="c")
                nc.vector.tensor_add(c[:], p0[:], p1[:])
                for c0 in range(lo, hi, 2048):
                    cbufs[(ti, c0)] = (c, lo)

            def emit_dve_silu(g, b, ti, lo, hi, last):
                c, chain_lo = cbufs[(ti, lo)]
                W = hi - lo
                y = yp.tile([128, W], bf16, tag="y")
                sw = 1024 if last else 2048
                for c0 in range(0, W, sw):
                    cw = min(sw, W - c0)
                    nc.scalar.activation(
                        out=y[:, c0 : c0 + cw],
                        in_=c[:, lo - chain_lo + c0 : lo - chain_lo + c0 + cw],
                        func=mybir.ActivationFunctionType.Silu,
                    )
                if last:
                    nc.scalar.dma_start(out=out_ap[g, :, b, lo:hi], in_=y[:])
                else:
                    nc.gpsimd.dma_start(out=out_ap[g, :, b, lo:hi], in_=y[:])

            for ui, (_, kind, ti, lo, hi) in enumerate(units):
                g, b = divmod(ti, B)
                last = ui == len(units) - 1
                if kind == "pe":
                    emit_pe(g, b, xts[ti], lo, hi, last)
                elif kind == "dvec":
                    emit_dve_chain(g, b, ti, xts[ti], lo, hi)
                else:
                    emit_dve_silu(g, b, ti, lo, hi, last)

    nc.compile()
    return nc


def _get_nc():
    if "nc" not in _CACHE:
        _CACHE["nc"] = _build()
    return _CACHE["nc"]


def _make_in_maps(x, w):
    x = np.asarray(x, dtype=np.float32)
    w = np.asarray(w, dtype=np.float32)

    # (B, S, D) -> (D, B, S), bf16, left-pad rows with PAD zeros.
    x_t = np.ascontiguousarray(x.transpose(2, 0, 1)).astype(ml_dtypes.bfloat16)
    x_pad = np.zeros((D, B, ROW), dtype=ml_dtypes.bfloat16)
    x_pad[:, :, PAD:] = x_t
    w_flat = np.ascontiguousarray(w[:, 0, :])  # (D, K) f32

    in_maps = []
    for i in range(N_CORES):
        lo, hi = i * D_LOCAL, (i + 1) * D_LOCAL
        m = {"x": np.ascontiguousarray(x_pad[lo:hi].reshape(G, 128, B, ROW))}
        m["w"] = np.ascontiguousarray(
            w_flat[lo:hi].reshape(G, 128, K).transpose(1, 0, 2).reshape(128, G * K)
        )
        # diag stationaries, laid out [128, G*K*128] partition-first
        wd = np.zeros((G, K, 128, 128), dtype=ml_dtypes.bfloat16)
        wl = w_flat[lo:hi].reshape(G, 128, K).astype(ml_dtypes.bfloat16)
        idx = np.arange(128)
        for g in range(G):
            for k in range(K):
                wd[g, k, idx, idx] = wl[g, :, k]
        # (G,K,p,m) -> (p, G,K,m) -> [128, G*K*128]
        m["wd"] = np.ascontiguousarray(
            wd.transpose(2, 0, 1, 3).reshape(128, G * K * 128)
        )
        in_maps.append(m)
    return in_maps


def _assemble(results):
    parts = []
    for r in results:
        y = np.asarray(r["out"]).reshape(D_LOCAL, B, S)
        parts.append(y)
    y_full = np.concatenate(parts, axis=0)  # (D, B, S) bf16
    return np.ascontiguousarray(y_full.transpose(1, 2, 0)).astype(np.float32)


def kernel(x, w):
    from concourse.bass_utils import run_bass_kernel_spmd

    nc = _get_nc()
    in_maps = _make_in_maps(x, w)
    trace = bool(int(os.environ.get("KERNEL_TRACE", "0")))
    res = None
    err = None
    for attempt in range(3):
        try:
            res = run_bass_kernel_spmd(
                nc, in_maps, core_ids=list(range(N_CORES)),
                trace=trace and attempt == 0,
            )
            break
        except Exception as e:  # transient NRT device errors / missing trace hook
            err = e
            os.environ["BASS_NEVER_TRACE"] = "1"
            trace = False
    if res is None:
        raise err
    _CACHE["last_results"] = res
    return _assemble(res.results)

